# revision 1
# baseline (speedup 1.0000x reference)
"""CTRNN Bass/Tile kernel for TRN2 (8-core data-parallel over batch).

Self-contained graded kernel: kernel(**inputs) -> (output [S,B,H] f32, h_last).

Math (per core, batch slice B=64):
    pre_t = a*(x_t @ W_in + b_in + b_hh) + h_{t-1} @ (a*W_hh)      [a = 0.2]
    h_t   = 0.8*h_{t-1} + relu(pre_t)
    out_t = h_t
Everything is computed in a transposed layout: h is stored as [H(part), B(free)]
split in two partition halves (H=256 = 2x128).  The per-step matmuls
accumulate into a PSUM slot that is pre-filled per 8-step chunk with the
x-projection (one N=512 GEMM per half) and the bias (K=1 ones-GEMM).  A
single custom DVE op then computes  h_new = 0.8*h_old + relu(psum)  in one
pass per step.
"""

import numpy as np
import ml_dtypes

import concourse.bass as bass
import concourse.bacc as bacc
import concourse.mybir as mybir
import concourse.tile as tile
import concourse.dve_ops as dve_ops
from concourse.dve_spec import Spec, Src0, Src1, C0, relu, lower
from concourse.dve_uop import DveOpSpec

BF = mybir.dt.bfloat16
F32 = mybir.dt.float32
NPBF = ml_dtypes.bfloat16

SEQ, BATCH, IN, HID = 1024, 512, 128, 256
ALPHA = 0.2
NCORES = 8
BLOC = BATCH // NCORES  # 64
DEBUG_FOLLOW = None


def _register_op():
    """Register the fused CTRNN blend op:  out = in0*s0 + relu(in1)."""
    name = "CTRNN_BLEND_ANT"
    for o in dve_ops.OPS:
        if o.name == name:
            return o

    def _ref(in0, in1, s0, s1, imm2):
        shape = in0.shape
        a = in0.astype(np.float32).reshape(shape[0], -1)
        b = in1.astype(np.float32).reshape(shape[0], -1)
        r = np.maximum(
            np.nan_to_num(b, nan=0.0, posinf=np.inf, neginf=-np.inf), 0
        )
        return (a * s0 + r).reshape(shape)

    spec = Spec(body=Src0 * C0 + relu(Src1), reference=_ref)
    idx = dve_ops._CUSTOM_DVE_ROW_BASE + len(dve_ops.OPS)
    assert idx < 0x20
    dve_ops._SUB_OPCODE_FOR_NAME[name] = idx
    shas = {}
    for ver in ("v3", "v4"):
        try:
            tmp = DveOpSpec(name=name, opcode=idx, uops=lower(spec, ver=ver), rd1_en=True)
            shas[ver] = tmp.sha(ver)
        except Exception:
            pass
    op = dve_ops.DveOp(name, spec, subdim=False, uops_sha=shas)
    dve_ops.OPS.append(op)
    dve_ops.CUSTOM_DVE_SPECS[name] = spec
    return op


def build_nc(S=SEQ, TC=8, TOUT=32, BLOC_=BLOC, NSPLIT=1):
    OP = _register_op()
    nc = bacc.Bacc()
    NCH = S // TC
    assert S % TC == 0 and S % TOUT == 0 and TOUT % TC == 0
    assert BLOC_ % NSPLIT == 0
    BC = BLOC_ // NSPLIT  # batch columns per chain

    xT = nc.declare_dram_parameter("xT", [128, S * BLOC_], BF, isOutput=False)
    h0 = nc.declare_dram_parameter("h0T", [128, 2 * BLOC_], BF, isOutput=False)
    wina = nc.declare_dram_parameter("wina", [128, 256], BF, isOutput=False)
    whha = nc.declare_dram_parameter("whha", [2, 128, 256], BF, isOutput=False)
    biasa = nc.declare_dram_parameter("biasa", [1, 256], BF, isOutput=False)
    outT = nc.declare_dram_parameter("outT", [128, 2 * S * BLOC_], BF, isOutput=True)

    with tile.TileContext(nc) as tc:
        with (
            tc.tile_pool(name="consts", bufs=1) as consts,
            tc.tile_pool(name="xin", bufs=3) as xin,
            tc.tile_pool(name="outp", bufs=2) as outp,
            tc.tile_pool(name="psum", bufs=2, space=bass.MemorySpace.PSUM) as psump,
        ):
            wina_sb = consts.tile([128, 256], BF)
            nc.sync.dma_start(out=wina_sb[:], in_=wina[:])
            whh0_sb = consts.tile([128, 256], BF, tag="whh0")
            nc.sync.dma_start(out=whh0_sb[:], in_=whha[0])
            whh1_sb = consts.tile([128, 256], BF, tag="whh1")
            nc.sync.dma_start(out=whh1_sb[:], in_=whha[1])
            biasa_sb = consts.tile([1, 256], BF)
            nc.sync.dma_start(out=biasa_sb[:], in_=biasa[:])
            h0_sb = consts.tile([128, 2, BLOC_], BF)
            nc.sync.dma_start(
                out=h0_sb[:].rearrange("p h b -> p (h b)"), in_=h0[:]
            )
            ones_sb = consts.tile([1, 512], BF)
            nc.vector.memset(ones_sb[:], 1.0)

            xts = {}
            pss = {}
            out_tiles = {}

            def emit_xdma(c):
                if c >= NCH:
                    return
                xt = xin.tile([128, TC * BLOC_], BF, tag="xt", name=f"xt{c}")
                # absorber: the direct2d DMA struct fits only one sem wait;
                # this 1-elem memset takes the slot-reuse waits so the DMA
                # below (same engine, in-order) needs none.
                nc.gpsimd.memset(xt[0:1, 0:1], 0.0)
                nc.gpsimd.dma_start(
                    out=xt[:],
                    in_=xT[:, c * TC * BLOC_ : (c + 1) * TC * BLOC_],
                )
                xts[c] = xt

            PSW = max(256, TC * BC)  # per-half column span inside the tile

            def xgemm_pieces(c):
                """Allocate chunk c's per-chain psum tiles and yield the
                x-projection matmuls one at a time so they can be spread
                across the previous chunk's steps (avoids a PE burst that
                head-of-line blocks the recurrence)."""
                if c >= NCH:
                    return
                xt = xts.pop(c)
                xtv = xt[:].rearrange("p (t b) -> p t b", t=TC)
                tiles = []
                for j in range(NSPLIT):
                    # one PSUM bank per chain when TC*BC <= 256 (both halves
                    # packed into a single bank); separate tiles per chain:
                    # Tile dep tracking is tensor-granular, so any shared
                    # tile would serialize chains
                    ps = psump.tile(
                        [128, 2, PSW], F32, tag=f"ps{j}", name=f"ps{c}_{j}"
                    )
                    tiles.append(ps)
                pss[c] = tiles
                for j in range(NSPLIT):
                    ps = tiles[j]
                    for m in (0, 1):
                        # start=True clears the WHOLE bank; only the first
                        # matmul per chain-chunk may use it (both halves
                        # share one bank) — the clear zero-fills half 1's
                        # region, which the m=1 matmul then accumulates into
                        yield lambda ps=ps, m=m, j=j: nc.tensor.matmul(
                            ps[:, m, 0 : TC * BC],
                            wina_sb[:, m * 128 : (m + 1) * 128],
                            xtv[:, :, j * BC : (j + 1) * BC],
                            start=(m == 0),
                            stop=False,
                            skip_group_check=True,
                        )
                        yield lambda ps=ps, m=m, j=j: nc.tensor.matmul(
                            ps[:, m, 0 : TC * BC],
                            biasa_sb[0:1, m * 128 : (m + 1) * 128],
                            ones_sb[0:1, 0 : TC * BC],
                            start=False,
                            stop=False,
                            skip_group_check=True,
                        )

            def emit_xgemm(c):
                for thunk in xgemm_pieces(c):
                    thunk()

            emit_xdma(0)
            emit_xdma(1)
            emit_xgemm(0)
            pending_pieces = []

            # one independent recurrence chain per batch sub-slice
            hprev_aps = [
                h0_sb[:, :, j * BC : (j + 1) * BC] for j in range(NSPLIT)
            ]

            for c in range(NCH):
                psl = pss.pop(c)
                for tau in range(TC):
                    t = c * TC + tau
                    oc, slot = divmod(t, TOUT)
                    if slot == 0:
                        out_tiles[oc] = [
                            outp.tile(
                                [128, 2, TOUT, BC], BF,
                                tag=f"out{j}", name=f"out{oc}_{j}",
                            )
                            for j in range(NSPLIT)
                        ]
                    last = tau == TC - 1
                    for j in range(NSPLIT):
                        ps = psl[j]
                        ot = out_tiles[oc][j]
                        sl = slice(tau * BC, (tau + 1) * BC)
                        hp = hprev_aps[j]
                        h0ap = hp[:, 0, :]
                        h1ap = hp[:, 1, :]
                        nc.tensor.matmul(
                            ps[:, 0, sl], whh0_sb[:, 0:128], h0ap, start=False, stop=False, skip_group_check=True
                        )
                        nc.tensor.matmul(
                            ps[:, 0, sl], whh1_sb[:, 0:128], h1ap, start=False, stop=last, skip_group_check=True
                        )
                        nc.tensor.matmul(
                            ps[:, 1, sl], whh0_sb[:, 128:256], h0ap, start=False, stop=False, skip_group_check=True
                        )
                        nc.tensor.matmul(
                            ps[:, 1, sl], whh1_sb[:, 128:256], h1ap, start=False, stop=last, skip_group_check=True
                        )
                        nc.vector._custom_dve(
                            OP,
                            out=ot[:, :, slot, :],
                            in0=hp,
                            in1=ps[:, :, sl],
                            s0=0.8,
                        )
                        hprev_aps[j] = ot[:, :, slot, :]
                    if tau == 0:
                        emit_xdma(c + 2)
                        pending_pieces = list(xgemm_pieces(c + 1))
                    # spread the next chunk's x-projection matmuls evenly
                    n_per_tau = (len(pending_pieces) + TC - 1) // TC if pending_pieces else 0
                    for _ in range(n_per_tau):
                        if pending_pieces:
                            pending_pieces.pop(0)()
                    if slot == TOUT - 1:
                        for j in range(NSPLIT):
                            otj = out_tiles[oc][j]
                            for m in (0, 1):
                                base = ((m * NSPLIT + j) * S + oc * TOUT) * BC
                                nc.gpsimd.dma_start(
                                    out=outT[:, base : base + TOUT * BC],
                                    in_=otj[:, m, :, :].rearrange(
                                        "p t b -> p (t b)"
                                    ),
                                )
                        del out_tiles[oc]
    nc.compile()
    return nc


# ---------------------------------------------------------------- host side


def prep_inputs(input, hidden, W_in, b_in, W_hh, b_hh, S=SEQ):
    """Shard + lay out host-side inputs for the 8 cores."""
    wina = (ALPHA * np.asarray(W_in)).astype(NPBF)
    whh = (ALPHA * np.asarray(W_hh)).astype(NPBF)
    whha = np.ascontiguousarray(whh.reshape(2, 128, 256))
    biasa = (ALPHA * (np.asarray(b_in) + np.asarray(b_hh))).reshape(1, 256).astype(NPBF)
    in_maps = []
    x = np.asarray(input)
    h = np.asarray(hidden)
    for c in range(NCORES):
        xc = np.ascontiguousarray(
            x[:S, c * BLOC : (c + 1) * BLOC, :].transpose(2, 0, 1)
        ).astype(NPBF).reshape(128, -1)  # [IN, S*B]
        hc = (
            np.ascontiguousarray(
                h[c * BLOC : (c + 1) * BLOC, :].T.reshape(2, 128, BLOC).transpose(1, 0, 2)
            )
        ).astype(NPBF).reshape(128, -1)  # [128, 2*B]
        in_maps.append(
            {"xT": xc, "h0T": hc, "wina": wina, "whha": whha, "biasa": biasa}
        )
    return in_maps


def assemble_output(results, S=SEQ, NSPLIT=1):
    BC = BLOC // NSPLIT
    outs = []
    for c in range(NCORES):
        # DRAM layout per core: [p, (m, j, t, bc)] chain-major
        o = np.asarray(results[c]["outT"]).reshape(128, 2, NSPLIT, S, BC)
        # -> [S, b = j*BC+bc, h = m*128+p]
        oc = (
            o.transpose(3, 2, 4, 1, 0)
            .reshape(S, BLOC, 256)
            .astype(np.float32)
        )
        outs.append(oc)
    output = np.concatenate(outs, axis=1)
    return output, output[-1].copy()


_NC_CACHE = {}


def _run_pjrt(nc, in_maps, n_iters=1, inner=1):
    """Execute the Bass module on the 8 axon cores via PJRT.

    Mirrors concourse.bass2jax.run_bass_via_pjrt (multi-core path) but
    without output-buffer donation so the jitted callable can be invoked
    repeatedly with device-resident inputs for steady-state timing.
    Returns (results, times_s): per-core output dicts + per-call wall times.
    """
    import time as _time
    import jax
    from jax.sharding import Mesh, PartitionSpec
    from jax.experimental.shard_map import shard_map
    import concourse.mybir as mb
    from concourse.bass2jax import (
        install_neuronx_cc_hook,
        _bass_exec_p,
        partition_id_tensor,
    )

    install_neuronx_cc_hook()
    n_cores = len(in_maps)
    partition_name = (
        nc.partition_id_tensor.name if nc.partition_id_tensor else None
    )

    in_names, out_names, out_avals, zero_outs = [], [], [], []
    for alloc in nc.m.functions[0].allocations:
        if not isinstance(alloc, mb.MemoryLocationSet):
            continue
        name = alloc.memorylocations[0].name
        if alloc.kind == "ExternalInput":
            if name != partition_name:
                in_names.append(name)
        elif alloc.kind == "ExternalOutput":
            out_names.append(name)
            shape = tuple(alloc.tensor_shape)
            dtype = mb.dt.np(alloc.dtype)
            out_avals.append(jax.core.ShapedArray(shape, dtype))
            zero_outs.append(np.zeros(shape, dtype))
    n_params = len(in_names)
    in_names_all = in_names + out_names
    if partition_name is not None:
        in_names_all.append(partition_name)

    def _body(*args):
        operands = list(args)
        if partition_name is not None:
            operands.append(partition_id_tensor())
        outs = None
        for _ in range(inner):
            outs = _bass_exec_p.bind(
                *operands,
                out_avals=tuple(out_avals),
                in_names=tuple(in_names_all),
                out_names=tuple(out_names),
                lowering_input_output_aliases=(),
                sim_require_finite=True,
                sim_require_nnan=True,
                nc=nc,
            )
        return tuple(outs)

    devices = jax.devices()[:n_cores]
    mesh = Mesh(np.asarray(devices), ("core",))
    in_specs = (PartitionSpec("core"),) * (n_params + len(out_names))
    out_specs = (PartitionSpec("core"),) * len(out_names)
    fn = jax.jit(
        shard_map(
            _body, mesh=mesh, in_specs=in_specs, out_specs=out_specs,
            check_rep=False,
        ),
        keep_unused=True,
    )
    concat_in = [
        np.concatenate([np.asarray(in_maps[c][nm]) for c in range(n_cores)], axis=0)
        for nm in in_names
    ]
    concat_zeros = [
        np.zeros((n_cores * z.shape[0], *z.shape[1:]), z.dtype) for z in zero_outs
    ]
    sharding = jax.sharding.NamedSharding(mesh, PartitionSpec("core"))
    dev_in = [jax.device_put(a, sharding) for a in concat_in]
    dev_zero = [jax.device_put(a, sharding) for a in concat_zeros]

    times = []
    outs = None
    for _ in range(max(1, n_iters)):
        t0 = _time.perf_counter()
        outs = fn(*dev_in, *dev_zero)
        jax.block_until_ready(outs)
        times.append(_time.perf_counter() - t0)

    results = []
    np_outs = [np.asarray(o) for o in outs]
    for c in range(n_cores):
        d = {}
        for i, nm in enumerate(out_names):
            per = np_outs[i].shape[0] // n_cores
            d[nm] = np_outs[i][c * per : (c + 1) * per]
        results.append(d)
    return results, times


NSPLIT_DEFAULT = 2


def kernel(input, hidden, W_in, b_in, W_hh, b_hh, n_iters=1):
    key = ("full", NSPLIT_DEFAULT)
    if key not in _NC_CACHE:
        _NC_CACHE[key] = build_nc(NSPLIT=NSPLIT_DEFAULT)
    nc = _NC_CACHE[key]
    in_maps = prep_inputs(input, hidden, W_in, b_in, W_hh, b_hh)
    results, times = _run_pjrt(nc, in_maps, n_iters=n_iters)
    out, h_last = assemble_output(results, NSPLIT=NSPLIT_DEFAULT)
    kernel.last_times = times
    kernel.last_exec_time_ns = int(min(times) * 1e9)
    return out, h_last
